# revision 9
# baseline (speedup 1.0000x reference)
"""Trainium2 Bass kernel for nn_Conv_agg_raw (GNN message passing).

Math: out = sum_k weight[k] @ (h @ resx[k]) + bias, where resx[k] is the
dense [N,N] scatter of edge features X[:,k] at (src,dst).  Equivalently
    res_k[:, m] = sum_{e: dst_e=m} X[e,k] * h[:, src_e]
    out[:, m]   = sum_k weight[k] @ res_k[:, m] + bias
We shard dst nodes across the 8 cores (512 each) - fully independent, no
collectives.  The host pre-gathers each core's per-edge h rows into an
edge-slot-ordered tensor quantized to fp8-e3m4 (hg8); the device streams
it sequentially (half the bytes of an fp16 gather, and sequential DMA
avoids the sub-512B descriptor penalty), aggregates edges into res via PE
matmuls (f8e3 stationary x f16 moving), then applies the stacked weight.
The weight is split into two column-halves (wt0/wt1) slotted mid-stream
so each output-half final can start as soon as its half arrives.
"""

import time
import numpy as np
import ml_dtypes

import concourse.bass as bass
import concourse.bacc as bacc
import concourse.tile as tile
from concourse import mybir
from concourse.bass_utils import run_bass_kernel_spmd

import os as _os

N = 4096
K = 8
C = 256
NCORES = 8
DSTS_PER_CORE = N // NCORES      # 512
SLOTS = int(_os.environ.get("GNN_SLOTS", "8"))   # dst slots per window
WINDOWS = DSTS_PER_CORE // SLOTS                 # windows per core
KS = K * SLOTS                                   # psum cols per window
GG = int(_os.environ.get("GNN_GG", "8"))         # windows per load group
G0 = [int(x) for x in _os.environ.get("GNN_G0", "2,6").split(",")]
NWU = int(_os.environ.get("GNN_NWU", "150"))     # PE pre-stream fillers
TAILF = int(_os.environ.get("GNN_TAILF", "20"))
FILLS = [int(x) for x in _os.environ.get(
    "GNN_FILLS", "0,60,40,40,40,20,0,0,0,0,0,0").split(",")]
WT0G = int(_os.environ.get("GNN_WT0G", "2"))     # load wT-half0 before group
WT1G = int(_os.environ.get("GNN_WT1G", "3"))     # load wT-half1 before group
RHSPOOL = int(_os.environ.get("GNN_RHSPOOL", "0"))  # every n-th rhs on Pool
COPYENG = _os.environ.get("GNN_COPYENG", "pool_act")
BIASENG = _os.environ.get("GNN_BIASENG", "act")
BANKW = 512 // KS                                # windows per psum bank
PRE_W = sum(G0)                                  # windows in the epk prefix

_prog_cache: dict = {}


# ---------------------------------------------------------------- device ----
def _build_program(caps):
    """Build the per-core Bass program.

    caps[w] = number of 128-edge chunks the w-th window holds (same vector
    for every core; the host packs each core's dsts to fit it)."""
    caps = list(caps)
    assert len(caps) == WINDOWS
    assert WINDOWS % GG == 0 and WINDOWS % BANKW == 0
    off = [0]
    for cw in caps:
        off.append(off[-1] + cw)
    nchunk = off[-1]                 # total chunks per core

    FSPLIT = 8
    nc = bacc.Bacc("TRN2", target_bir_lowering=False, debug=False)
    f32 = mybir.dt.float32
    f16 = mybir.dt.float16
    f8 = mybir.dt.float8e3
    i8 = mybir.dt.int8

    # pre-gathered per-edge h rows, e3m4 bytes: hg8[p, ch*C + c] is channel c
    # of the h row for edge slot j = ch*128 + p
    hg8_d = nc.dram_tensor("hg8", [128, nchunk * C], i8, kind="ExternalInput")
    # stacked weight halves, host-packed in SBUF layout [p, q*128 + c]
    wt0_d = nc.dram_tensor("wt0", [128, 16 * 128], f16, kind="ExternalInput")
    wt1_d = nc.dram_tensor("wt1", [128, 16 * 128], f16, kind="ExternalInput")
    # epk = [dl | iota | pad | biasT | xr] packed as one int16-typed tensor
    # (dl/iota/xr are fp16 bitcast, bias f32 bitcast -> 4 int16 cols).
    # dl/iota/bias + the first PRE_W windows' xr form a prefix load so the
    # mask + first rhs come up as early as possible.
    EPK_DL = nchunk
    EPK_IOTA = SLOTS
    EPK_PAD = (EPK_DL + EPK_IOTA) % 2
    XR_OFF = EPK_DL + EPK_IOTA + EPK_PAD + 4
    epk_cols = XR_OFF + nchunk * K
    epk_pre = XR_OFF + off[PRE_W] * K
    epk_d = nc.dram_tensor("epk", [128, epk_cols], mybir.dt.int16,
                           kind="ExternalInput")
    out_d = nc.dram_tensor("out", [128, FSPLIT * 128 + 64],
                           mybir.dt.float16, kind="ExternalOutput")

    FW = WINDOWS // FSPLIT           # windows per final group

    with tile.TileContext(nc) as tc:
        with (
            tc.tile_pool(name="persist", bufs=1) as pp,
            tc.tile_pool(name="hg", bufs=7) as hgp,
            tc.tile_pool(name="rhs", bufs=8) as rhp,
            tc.tile_pool(name="ps0", bufs=2, space="PSUM") as ps0,
            tc.tile_pool(name="ps1", bufs=2, space="PSUM") as ps1,
            tc.tile_pool(name="psf", bufs=3, space="PSUM") as psf,
            tc.tile_pool(name="psw", bufs=1, space="PSUM") as psw,
        ):
            # ---- epk prefix first: it gates the mask + first rhs build ----
            epk_sb = pp.tile([128, epk_cols], mybir.dt.int16)
            nc.sync.dma_start(epk_sb[:, :epk_pre], epk_d.ap()[:, :epk_pre])
            dl_sb = epk_sb[:, :EPK_DL].bitcast(f16)
            iota_sb = epk_sb[:, EPK_DL:EPK_DL + EPK_IOTA].bitcast(f16)
            bias_sb = epk_sb[:, EPK_DL + EPK_IOTA + EPK_PAD:
                             EPK_DL + EPK_IOTA + EPK_PAD + 4].bitcast(f32)
            xr_sb = epk_sb[:, XR_OFF:].bitcast(f16)

            # ---- dst-slot one-hot mask: mask[p, ch, d] = (dl[p, ch] == d)
            # prefix chunks first (gates rhs0), then the rest
            mask_all = pp.tile([128, nchunk * SLOTS], f16)
            mview = mask_all[:].rearrange("p (ch d) -> p ch d", d=SLOTS)
            for lo, hi in ((0, off[PRE_W]), (off[PRE_W], nchunk)):
                nc.vector.tensor_tensor(
                    mview[:, lo:hi],
                    dl_sb[:, lo:hi].unsqueeze(-1)
                    .broadcast_to([128, hi - lo, SLOTS]),
                    iota_sb.unsqueeze(1).broadcast_to([128, hi - lo, SLOTS]),
                    mybir.AluOpType.is_equal,
                )

            # res accumulator (SBUF, fp16): per c-half; col = w*KS + k*SLOTS + d
            resstack = [pp.tile([128, WINDOWS * KS], f16, name=f"resstack{i}")
                        for i in range(2)]
            # output staging: fh-major blocks of 128 cols (oh0|oh1), wu tail
            out_sb = pp.tile([128, FSPLIT * 128 + 64], f16, name="out_sb")

            # wt halves: wt_sbs[oh][p, q*128 + c] for q-chunk q, in-col c
            wt_sbs = [pp.tile([128, 16 * 128], f16, name=f"wt{i}")
                      for i in range(2)]

            # ---- PE warm-up: release the p-state throttle while the
            # prologue DMAs fill.
            wu = pp.tile([128, 128], f16, name="wu")
            nc.gpsimd.memset(wu[:], 0.0)
            wps = psw.tile([128, 32], f32, tag="psw", name="wps")

            fill_state = [False]

            def fill_pe(n, last=False):
                for _ in range(n):
                    nc.tensor.matmul(wps[:], wu[:], wu[:, :32],
                                     start=not fill_state[0],
                                     stop=False, skip_group_check=True)
                    fill_state[0] = True
                if last:
                    nc.tensor.matmul(wps[:], wu[:], wu[:, :32],
                                     start=not fill_state[0], stop=True,
                                     skip_group_check=True)

            fill_pe(NWU)

            biaseng = [0]
            tail_bias = [False]

            def emit_final(w_lo, w_cnt, co, oh):
                """Apply weight half oh to windows [w_lo, w_lo+w_cnt); result
                lands in out_sb cols [co + oh*m2, co + (oh+1)*m2)."""
                m2 = w_cnt * SLOTS
                pso = psf.tile([128, m2], f32, tag="psf", name="pso")
                for q in range(16):      # q = (k, ci_half)
                    k, cih = divmod(q, 2)
                    rs = resstack[cih][:]
                    rhs_ap = bass.AP(
                        rs.tensor,
                        rs.offset + w_lo * KS + k * SLOTS,
                        [[WINDOWS * KS, 128], [KS, w_cnt], [1, SLOTS]],
                    )
                    nc.tensor.matmul(
                        pso[:],
                        wt_sbs[oh][:, q * 128:(q + 1) * 128],
                        rhs_ap,
                        start=(q == 0),
                        stop=(q == 15),
                    )
                dst = out_sb[:, co + oh * m2:co + (oh + 1) * m2]
                use_dve = (BIASENG == "dve" or tail_bias[0]
                           or (BIASENG == "alt" and biaseng[0] % 2 == 0))
                if use_dve:
                    nc.vector.tensor_tensor(
                        dst, pso[:],
                        bias_sb[:, oh:oh + 1].broadcast_to([128, m2]),
                        mybir.AluOpType.add)
                else:
                    nc.scalar.add(dst, pso[:], bias_sb[:, oh:oh + 1])
                biaseng[0] += 1

            # window groups: small starters for an early PE start, steady
            # 8-window groups, then a short tail so the last hg load covers a
            # single window
            GROUPS = []
            wdone = 0
            for g0 in G0:
                GROUPS.append((wdone, g0))
                wdone += g0
            while wdone + GG <= WINDOWS - GG:
                GROUPS.append((wdone, GG))
                wdone += GG
            GROUPS += [(wdone, 4), (wdone + 4, 3), (wdone + 7, 1)]
            # final segments: (w_lo, w_cnt, out_sb col offset)
            SEGS = [(i * FW, FW, i * 2 * FW * SLOTS)
                    for i in range(WINDOWS // FW - 1)]
            SEGS += [(WINDOWS - FW, FW - 1, SEGS[-1][2] + 2 * FW * SLOTS),
                     (WINDOWS - 1, 1, SEGS[-1][2] + 4 * FW * SLOTS
                      - 2 * SLOTS)]
            OUT_COLS = SEGS[-1][2] + 2 * SLOTS

            pbank = [None, None]
            fin_done = [0, 0]        # per-oh emission cursor
            wtg = [WT0G, WT1G]
            rhs_n = [0]
            for g, (w0, wcnt) in enumerate(GROUPS):
                if g == 1:
                    nc.sync.dma_start(epk_sb[:, epk_pre:],
                                      epk_d.ap()[:, epk_pre:])
                for oh in range(2):
                    if g == wtg[oh]:
                        nc.sync.dma_start(
                            wt_sbs[oh][:],
                            (wt0_d if oh == 0 else wt1_d).ap())
                # finals first so queued PE work covers DMA waits
                for oh in range(2):
                    while (g > wtg[oh] and fin_done[oh] < len(SEGS) - 2
                           and SEGS[fin_done[oh]][0]
                           + SEGS[fin_done[oh]][1] <= w0):
                        emit_final(*SEGS[fin_done[oh]], oh)
                        fin_done[oh] += 1
                if g < len(FILLS):
                    fill_pe(FILLS[g])
                # stream this group's pre-gathered h rows (f8e3 bytes)
                gch = off[w0 + wcnt] - off[w0]   # chunks in this group
                hg = hgp.tile([128, gch, C], mybir.dt.int8, tag="hg")
                nc.sync.dma_start(
                    hg[:],
                    hg8_d.ap()[:, off[w0] * C:off[w0 + wcnt] * C]
                    .rearrange("p (g c) -> p g c", c=C))
                # rhs for the whole group in one op:
                # rhs[p, ch, k, d] = xr[p, ch, k] * mask[p, ch, d]
                rhs = rhp.tile([128, gch, KS], f16, tag="rhs")
                xr_g = bass.AP(xr_sb.tensor,
                               xr_sb.offset + off[w0] * K,
                               [[epk_cols, 128], [K, gch],
                                [1, K], [0, SLOTS]])
                mk_g = bass.AP(mask_all[:].tensor,
                               mask_all[:].offset + off[w0] * SLOTS,
                               [[nchunk * SLOTS, 128], [SLOTS, gch],
                                [0, K], [1, SLOTS]])
                rview = rhs[:].rearrange("p ch (k d) -> p ch k d", d=SLOTS)
                if RHSPOOL > 0 and rhs_n[0] % RHSPOOL == RHSPOOL - 1:
                    nc.gpsimd.tensor_mul(rview, xr_g, mk_g)
                else:
                    nc.vector.tensor_tensor(rview, xr_g, mk_g,
                                            mybir.AluOpType.mult)
                rhs_n[0] += 1
                for wl in range(wcnt):
                    w = w0 + wl
                    cw = caps[w]
                    if w < WINDOWS - BANKW:
                        bslot = w % BANKW
                        if bslot == 0:
                            pbank[0] = ps0.tile([128, 512], f32, tag="b0",
                                                name="pb0")
                            pbank[1] = ps1.tile([128, 512], f32, tag="b1",
                                                name="pb1")
                    elif w == WINDOWS - BANKW:
                        bslot = 0
                        pbank[0] = ps0.tile([128, (BANKW - 1) * KS], f32,
                                            tag="b0", name="pb0a")
                        pbank[1] = ps1.tile([128, (BANKW - 1) * KS], f32,
                                            tag="b1", name="pb1a")
                    elif w < WINDOWS - 1:
                        bslot = w % BANKW
                    else:
                        bslot = 0
                        pbank[0] = ps0.tile([128, KS], f32, tag="b0",
                                            name="pb0b")
                        pbank[1] = ps1.tile([128, KS], f32, tag="b1",
                                            name="pb1b")
                    # aggregate: psum[ci, (k,d)] += Hg_chunk.T @ rhs_chunk
                    for ch in range(cw):
                        chl = off[w] - off[w0] + ch
                        for half in range(2):
                            nc.tensor.matmul(
                                pbank[half][:, bslot * KS:(bslot + 1) * KS],
                                hg[:, chl, half * 128:(half + 1) * 128]
                                .bitcast(f8),
                                rhs[:, chl, :],
                                start=(ch == 0),
                                stop=(ch == cw - 1),
                            )
                    # copy finished bank columns to resstack in two pieces
                    # per bank: the bulk early (after the next-to-last
                    # window), the last window as a tiny low-latency op.
                    copy_parts = []      # (w_lo, w_hi, bank_slot_lo)
                    if w < WINDOWS - BANKW:
                        if w % BANKW == BANKW - 2:
                            copy_parts = [(w - (BANKW - 2), w + 1, 0)]
                        elif w % BANKW == BANKW - 1:
                            copy_parts = [(w, w + 1, BANKW - 1)]
                    elif w == WINDOWS - 2:
                        copy_parts = [(WINDOWS - BANKW, w + 1, 0)]
                    elif w == WINDOWS - 1:
                        copy_parts = [(w, w + 1, 0)]
                    for (clo, chi, bslo) in copy_parts:
                        ncols = (chi - clo) * KS
                        for half in range(2):
                            dst = resstack[half][:, clo * KS:chi * KS]
                            srcb = pbank[half][:, bslo * KS:
                                               bslo * KS + ncols]
                            if half == 0:
                                nc.vector.tensor_copy(dst, srcb)
                            else:
                                nc.scalar.copy(dst, srcb)

            # store segs 0..-3 as soon as their finals are in out_sb
            nc.sync.dma_start(out_d.ap()[:, :SEGS[-2][2]],
                              out_sb[:, :SEGS[-2][2]])
            tail_bias[0] = True
            fill_pe(TAILF)
            for oh in range(2):
                emit_final(*SEGS[-2], oh)
            nc.sync.dma_start(
                out_d.ap()[:, SEGS[-2][2]:SEGS[-1][2]],
                out_sb[:, SEGS[-2][2]:SEGS[-1][2]])
            for oh in range(2):
                emit_final(*SEGS[-1], oh)
            fill_pe(0, last=True)
            nc.vector.tensor_copy(
                out_sb[:, OUT_COLS:OUT_COLS + 32].bitcast(f32),
                wps[:, :16])
            nc.sync.dma_start(out_d.ap()[:, SEGS[-1][2]:],
                              out_sb[:, SEGS[-1][2]:])

    nc.compile()
    return nc


# ------------------------------------------------------------------ host ----
def _greedy_partition(items_deg, nbins, cap):
    """Assign item ids (sorted desc by degree) to bins; each bin gets at most
    `cap` items, minimizing max degree-sum.  Returns list of lists."""
    import heapq
    bins = [[] for _ in range(nbins)]
    heap = [(0, b) for b in range(nbins)]
    heapq.heapify(heap)
    for it, dg in items_deg:
        s, b = heapq.heappop(heap)
        bins[b].append(it)
        if len(bins[b]) < cap:
            heapq.heappush(heap, (s + dg, b))
    return bins


def _pack_windows(items_deg, caps):
    """Pack (dst, deg) items into len(caps) bins of SLOTS items each with
    bin w's degree-sum <= caps[w]*128.  Returns list of lists or None."""
    nb = len(caps)
    rem_cap = [c * 128 for c in caps]
    rem_slots = [SLOTS] * nb
    bins = [[] for _ in range(nb)]
    for it, dg in items_deg:           # desc by degree
        best, best_score = -1, None
        for b in range(nb):
            if rem_slots[b] == 0 or rem_cap[b] < dg:
                continue
            score = (rem_cap[b] - dg) / rem_slots[b]
            if best_score is None or score > best_score:
                best, best_score = b, score
        if best < 0:
            return None
        bins[best].append(it)
        rem_cap[best] -= dg
        rem_slots[best] -= 1
    return bins


def _make_caps(E_core):
    """Per-window-ordinal chunk capacities.  Wide windows sit in the middle
    groups (absorbing degree-sum variance), the tail gets narrow windows so
    the last hg load covers a single window."""
    base = -(-E_core // (WINDOWS * 128))           # avg chunks per window
    outs = []
    for nwide in (1, 2, 3, 4, 6, 8, 16, 24, WINDOWS - GG):
        caps = ([base] * GG + [base + 1] * nwide
                + [base] * (WINDOWS - GG - nwide))
        outs.append(caps)
    return outs


def kernel(h, X, edge_index, batch_node, weight, bias):
    h = np.asarray(h, dtype=np.float32)
    X = np.asarray(X, dtype=np.float32)
    edge_index = np.asarray(edge_index)
    weight = np.asarray(weight, dtype=np.float32)
    bias = np.asarray(bias, dtype=np.float32)

    src = edge_index[0].astype(np.int64)
    dst = edge_index[1].astype(np.int64)
    E = src.shape[0]

    deg = np.bincount(dst, minlength=N)
    order = np.argsort(-deg, kind="stable")

    # dst -> core (8 bins of 512), then per core dst -> window
    core_bins = _greedy_partition([(int(m), int(deg[m])) for m in order],
                                  NCORES, DSTS_PER_CORE)

    # edges grouped by dst
    eorder = np.argsort(dst, kind="stable")
    starts = np.searchsorted(dst[eorder], np.arange(N))
    ends = np.searchsorted(dst[eorder], np.arange(N) + 1)

    core_windows = None
    caps = None
    for caps_try in _make_caps(E // NCORES):
        packs = []
        for c in range(NCORES):
            items = [(m, int(deg[m])) for m in
                     sorted(core_bins[c], key=lambda m: -deg[m])]
            p = _pack_windows(items, caps_try)
            if p is None:
                break
            packs.append(p)
        if len(packs) == NCORES:
            core_windows, caps = packs, caps_try
            break
    if core_windows is None:
        # fallback: uniform capacity from the worst window under plain LPT
        max_cnt = 0
        core_windows = []
        for c in range(NCORES):
            items = [(m, int(deg[m])) for m in
                     sorted(core_bins[c], key=lambda m: -deg[m])]
            wins = _greedy_partition(items, WINDOWS, SLOTS)
            core_windows.append(wins)
            for wlist in wins:
                max_cnt = max(max_cnt, int(sum(deg[m] for m in wlist)))
        caps = [max(1, -(-max_cnt // 128))] * WINDOWS

    key = (tuple(caps), SLOTS, GG, NWU, TAILF, WT0G, WT1G, RHSPOOL,
           COPYENG, BIASENG, tuple(G0), tuple(FILLS))
    if key not in _prog_cache:
        _prog_cache[key] = _build_program(caps)
    nc = _prog_cache[key]

    off = [0]
    for cw in caps:
        off.append(off[-1] + cw)
    nchunk = off[-1]
    # e3m4-quantized h rows, [N, C] bytes
    h8T = np.ascontiguousarray(h.T).astype(ml_dtypes.float8_e3m4) \
        .view(np.int8)
    wT = np.ascontiguousarray(
        weight.transpose(0, 2, 1).reshape(K * C, C)).astype(np.float16)
    wtq = wT.reshape(16, 128, 256).transpose(1, 0, 2)   # [p, q, c]
    wt0 = np.ascontiguousarray(wtq[:, :, :128].reshape(128, 16 * 128))
    wt1 = np.ascontiguousarray(wtq[:, :, 128:].reshape(128, 16 * 128))
    bias2 = np.ascontiguousarray(bias.reshape(2, 128))
    iota = np.broadcast_to(np.arange(SLOTS, dtype=np.float16),
                           (128, SLOTS)).copy()

    in_maps = []
    perms = []
    for c in range(NCORES):
        hg8 = np.zeros((128, nchunk, C), dtype=np.int8)
        xr = np.zeros((128, nchunk, K), dtype=np.float16)
        dl = np.zeros((128, nchunk), dtype=np.float16)
        perm = np.empty(DSTS_PER_CORE, dtype=np.int64)
        for w in range(WINDOWS):
            wl = core_windows[c][w]
            el = []
            sl = []
            for d_slot, m in enumerate(wl):
                perm[w * SLOTS + d_slot] = m
                ee = eorder[starts[m]:ends[m]]
                el.append(ee)
                sl.append(np.full(ee.shape[0], d_slot, dtype=np.float16))
            el = (np.concatenate(el) if el else
                  np.empty(0, dtype=np.int64))
            sl = (np.concatenate(sl) if sl else
                  np.empty(0, dtype=np.float16))
            so = np.argsort(src[el], kind="stable")
            el, sl = el[so], sl[so]
            L = el.shape[0]
            j = np.arange(L)
            p = j % 128
            ch = off[w] + j // 128
            xr[p, ch, :] = X[el, :]
            dl[p, ch] = sl
            hg8[p, ch, :] = h8T[src[el], :]
        pad = np.zeros((128, (nchunk + SLOTS) % 2), dtype=np.int16)
        epk = np.concatenate(
            [dl.view(np.int16),
             iota.view(np.int16), pad,
             np.ascontiguousarray(bias2.T.astype(np.float32))
               .view(np.int16).reshape(128, 4),
             xr.reshape(128, nchunk * K).view(np.int16)],
            axis=1)
        in_maps.append({
            "hg8": np.ascontiguousarray(hg8.reshape(128, nchunk * C)),
            "wt0": wt0,
            "wt1": wt1,
            "epk": np.ascontiguousarray(epk),
        })
        perms.append(perm)

    global _last_perms
    _last_perms = perms

    try:
        res = run_bass_kernel_spmd(nc, in_maps, core_ids=list(range(NCORES)))
    except Exception:
        # transient device-state issues usually clear on retry
        time.sleep(10)
        res = run_bass_kernel_spmd(nc, in_maps, core_ids=list(range(NCORES)))

    FW = 8
    segs = [(i * FW, FW, i * 2 * FW * SLOTS)
            for i in range(WINDOWS // FW - 1)]
    segs += [(WINDOWS - FW, FW - 1, segs[-1][2] + 2 * FW * SLOTS),
             (WINDOWS - 1, 1, segs[-1][2] + 4 * FW * SLOTS - 2 * SLOTS)]
    out = np.empty((C, N), dtype=np.float32)
    for c in range(NCORES):
        raw = np.asarray(res.results[c]["out"]).astype(np.float32)
        oc = np.empty((C, DSTS_PER_CORE), dtype=np.float32)
        for w_lo, w_cnt, co in segs:
            m2 = w_cnt * SLOTS
            for oh in range(2):
                oc[oh * 128:(oh + 1) * 128,
                   w_lo * SLOTS:w_lo * SLOTS + m2] = \
                    raw[:, co + oh * m2:co + (oh + 1) * m2]
        out[:, perms[c]] = oc
    return out


# revision 10
# speedup vs baseline: 1.1432x; 1.1432x over previous
"""Trainium2 Bass kernel for nn_Conv_agg_raw (GNN message passing).

Math: out = sum_k weight[k] @ (h @ resx[k]) + bias, where resx[k] is the
dense [N,N] scatter of edge features X[:,k] at (src,dst).  Equivalently
    res_k[:, m] = sum_{e: dst_e=m} X[e,k] * h[:, src_e]
    out[:, m]   = sum_k weight[k] @ res_k[:, m] + bias
We shard dst nodes across the 8 cores (512 each) - fully independent, no
collectives.  The host pre-gathers each core's per-edge h rows into an
edge-slot-ordered tensor quantized to fp8-e3m4 (hg8); the device streams
it sequentially (half the bytes of an fp16 gather, and sequential DMA
avoids the sub-512B descriptor penalty), aggregates edges into res via PE
matmuls (f8e3 stationary x f16 moving), then applies the stacked weight.
"""

import time
import numpy as np
import ml_dtypes

import concourse.bass as bass
import concourse.bacc as bacc
import concourse.tile as tile
from concourse import mybir
from concourse.bass_utils import run_bass_kernel_spmd

import os as _os

N = 4096
K = 8
C = 256
NCORES = 8
DSTS_PER_CORE = N // NCORES      # 512
SLOTS = int(_os.environ.get("GNN_SLOTS", "8"))   # dst slots per window
WINDOWS = DSTS_PER_CORE // SLOTS                 # windows per core
KS = K * SLOTS                                   # psum cols per window
GG = int(_os.environ.get("GNN_GG", "8"))         # windows per load group
NWU = int(_os.environ.get("GNN_NWU", "280"))     # PE pre-stream fillers
TAILF = int(_os.environ.get("GNN_TAILF", "30"))
FILLS = [int(x) for x in _os.environ.get(
    "GNN_FILLS", "0,0,0,100,0,0,0,0,0,0").split(",")]
WTG = int(_os.environ.get("GNN_WTG", "3"))       # load wT before this group
HGDT = _os.environ.get("GNN_HGDT", "f8")         # hg dtype: f8 | f16
BANKW = 512 // KS                                # windows per psum bank

_prog_cache: dict = {}


# ---------------------------------------------------------------- device ----
def _build_program(caps):
    """Build the per-core Bass program.

    caps[w] = number of 128-edge chunks the w-th window holds (same vector
    for every core; the host packs each core's dsts to fit it)."""
    caps = list(caps)
    assert len(caps) == WINDOWS
    assert WINDOWS % GG == 0 and WINDOWS % BANKW == 0
    off = [0]
    for cw in caps:
        off.append(off[-1] + cw)
    nchunk = off[-1]                 # total chunks per core

    FSPLIT = 8
    nc = bacc.Bacc("TRN2", target_bir_lowering=False, debug=False)
    f32 = mybir.dt.float32
    f16 = mybir.dt.float16
    f8 = mybir.dt.float8e3
    i8 = mybir.dt.int8

    # pre-gathered per-edge h rows: hg8[p, ch*C + c] is channel c of the
    # h row for edge slot j = ch*128 + p (e3m4 bytes, or f16 when HGDT=f16)
    HGB = 1 if HGDT == "f8" else 2     # bytes per element
    hg_store = i8 if HGDT == "f8" else mybir.dt.int16
    hg_view = f8 if HGDT == "f8" else f16
    hg8_d = nc.dram_tensor("hg8", [128, nchunk * C], hg_store,
                           kind="ExternalInput")
    wT = nc.dram_tensor("wT", [K * C, C], f16, kind="ExternalInput")
    # epk = [xr | dl | iota | biasT] packed as one int16-typed tensor
    # (xr/dl/iota are fp16 bitcast, bias f32 bitcast -> 4 int16 cols)
    EPK_XR = nchunk * K
    EPK_DL = nchunk
    EPK_IOTA = SLOTS
    EPK_PAD = (EPK_XR + EPK_DL + EPK_IOTA) % 2
    epk_cols = EPK_XR + EPK_DL + EPK_IOTA + EPK_PAD + 4
    epk_d = nc.dram_tensor("epk", [128, epk_cols], mybir.dt.int16,
                           kind="ExternalInput")
    out_d = nc.dram_tensor("out", [128, FSPLIT * 128 + 64],
                           mybir.dt.float16, kind="ExternalOutput")

    FW = WINDOWS // FSPLIT           # windows per final group
    M2 = FW * SLOTS                  # cols per final matmul

    with tile.TileContext(nc) as tc:
        with (
            tc.tile_pool(name="persist", bufs=1) as pp,
            tc.tile_pool(name="hg", bufs=5) as hgp,
            tc.tile_pool(name="rhs", bufs=4) as rhp,
            tc.tile_pool(name="ps0", bufs=2, space="PSUM") as ps0,
            tc.tile_pool(name="ps1", bufs=2, space="PSUM") as ps1,
            tc.tile_pool(name="psf", bufs=3, space="PSUM") as psf,
            tc.tile_pool(name="psw", bufs=1, space="PSUM") as psw,
        ):
            # ---- epk first: it gates the mask + first rhs build ----
            epk_sb = pp.tile([128, epk_cols], mybir.dt.int16)
            nc.sync.dma_start(epk_sb[:], epk_d.ap())
            xr_sb = epk_sb[:, :EPK_XR].bitcast(f16)
            dl_sb = epk_sb[:, EPK_XR:EPK_XR + EPK_DL].bitcast(f16)
            iota_sb = epk_sb[:, EPK_XR + EPK_DL:
                             EPK_XR + EPK_DL + EPK_IOTA].bitcast(f16)
            bias_sb = epk_sb[:, EPK_XR + EPK_DL + EPK_IOTA
                             + EPK_PAD:].bitcast(f32)

            # ---- dst-slot one-hot mask for every chunk: one big DVE op ----
            # mask_all[p, ch, d] = (dl[p, ch] == d)   (fp16 in+out -> 2x DVE)
            mask_all = pp.tile([128, nchunk * SLOTS], f16)
            nc.vector.tensor_tensor(
                mask_all[:].rearrange("p (ch d) -> p ch d", d=SLOTS),
                dl_sb.unsqueeze(-1).broadcast_to([128, nchunk, SLOTS]),
                iota_sb.unsqueeze(1).broadcast_to([128, nchunk, SLOTS]),
                mybir.AluOpType.is_equal,
            )

            # res accumulator (SBUF, fp16): per c-half; col = w*KS + k*SLOTS + d
            resstack = [pp.tile([128, WINDOWS * KS], f16, name=f"resstack{i}")
                        for i in range(2)]
            # output staging: fh-major blocks of 128 cols (oh0|oh1), wu tail
            out_sb = pp.tile([128, FSPLIT * 128 + 64], f16, name="out_sb")

            wt_sb = pp.tile([128, 16 * C], f16)   # chunk q at cols [q*256,..)

            # ---- PE warm-up: release the p-state throttle while the
            # prologue DMAs fill.  Lands in psw (drained into out_sb's junk
            # tail later) so dead-code passes keep the chain.
            wu = pp.tile([128, 128], f16, name="wu")
            nc.gpsimd.memset(wu[:], 0.0)
            wps = psw.tile([128, 32], f32, tag="psw", name="wps")

            fill_state = [False]

            def fill_pe(n, last=False):
                for _ in range(n):
                    nc.tensor.matmul(wps[:], wu[:], wu[:, :32],
                                     start=not fill_state[0],
                                     stop=False, skip_group_check=True)
                    fill_state[0] = True
                if last:
                    nc.tensor.matmul(wps[:], wu[:], wu[:, :32],
                                     start=not fill_state[0], stop=True,
                                     skip_group_check=True)

            fill_pe(NWU)

            def emit_final(w_lo, w_cnt, co, add_eng="dve"):
                """Apply stacked weight to windows [w_lo, w_lo+w_cnt); the
                result lands in out_sb cols [co, co + 2*w_cnt*SLOTS)."""
                m2 = w_cnt * SLOTS
                for oh in range(2):
                    pso = psf.tile([128, m2], f32, tag="psf", name="pso")
                    for q in range(16):      # q = (k, ci_half)
                        k, cih = divmod(q, 2)
                        rs = resstack[cih][:]
                        rhs_ap = bass.AP(
                            rs.tensor,
                            rs.offset + w_lo * KS + k * SLOTS,
                            [[WINDOWS * KS, 128], [KS, w_cnt], [1, SLOTS]],
                        )
                        nc.tensor.matmul(
                            pso[:],
                            wt_sb[:, q * 256 + oh * 128:
                                  q * 256 + oh * 128 + 128],
                            rhs_ap,
                            start=(q == 0),
                            stop=(q == 15),
                        )
                    dst = out_sb[:, co + oh * m2:co + (oh + 1) * m2]
                    eng = add_eng if add_eng != "mix" else \
                        ("act" if oh == 0 else "dve")
                    if eng == "dve":
                        nc.vector.tensor_tensor(
                            dst, pso[:],
                            bias_sb[:, oh:oh + 1].broadcast_to([128, m2]),
                            mybir.AluOpType.add)
                    else:
                        nc.scalar.add(dst, pso[:], bias_sb[:, oh:oh + 1])

            # window groups: steady 8-window groups, then a short tail so the
            # last hg load covers a single window
            GROUPS = []
            wdone = 0
            while wdone + GG <= WINDOWS - GG:
                GROUPS.append((wdone, GG))
                wdone += GG
            GROUPS += [(wdone, 4), (wdone + 4, 3), (wdone + 7, 1)]
            # final segments: (w_lo, w_cnt, out_sb col offset)
            SEGS = [(i * FW, FW, i * 2 * FW * SLOTS)
                    for i in range(WINDOWS // FW - 1)]
            SEGS += [(WINDOWS - FW, FW - 1, SEGS[-1][2] + 2 * FW * SLOTS),
                     (WINDOWS - 1, 1, SEGS[-1][2] + 4 * FW * SLOTS
                      - 2 * SLOTS)]
            OUT_COLS = SEGS[-1][2] + 2 * SLOTS

            pbank = [None, None]
            fin_done = 0
            if WTG <= 0:
                nc.sync.dma_start(
                    wt_sb[:], wT.ap().rearrange("(q p) c -> p q c", p=128))
            for g, (w0, wcnt) in enumerate(GROUPS):
                if WTG > 0 and g == WTG:
                    # wT load slotted into the stream after a few hg groups
                    # so PE has queued agg work to chew through meanwhile
                    nc.sync.dma_start(
                        wt_sb[:], wT.ap().rearrange("(q p) c -> p q c",
                                                    p=128))
                if g < len(FILLS):
                    fill_pe(FILLS[g])
                # stream this group's pre-gathered h rows (f8e3 bytes)
                gch = off[w0 + wcnt] - off[w0]   # chunks in this group
                hg = hgp.tile([128, gch, C], hg_store, tag="hg")
                nc.sync.dma_start(
                    hg[:],
                    hg8_d.ap()[:, off[w0] * C:off[w0 + wcnt] * C]
                    .rearrange("p (g c) -> p g c", c=C))
                # rhs for the whole group in one DVE op:
                # rhs[p, ch, k, d] = xr[p, ch, k] * mask[p, ch, d]
                rhs = rhp.tile([128, gch, KS], f16, tag="rhs")
                xr_g = bass.AP(xr_sb.tensor,
                               xr_sb.offset + off[w0] * K,
                               [[epk_cols, 128], [K, gch],
                                [1, K], [0, SLOTS]])
                mk_g = bass.AP(mask_all[:].tensor,
                               mask_all[:].offset + off[w0] * SLOTS,
                               [[nchunk * SLOTS, 128], [SLOTS, gch],
                                [0, K], [1, SLOTS]])
                nc.vector.tensor_tensor(
                    rhs[:].rearrange("p ch (k d) -> p ch k d", d=SLOTS),
                    xr_g, mk_g, mybir.AluOpType.mult,
                )
                for wl in range(wcnt):
                    w = w0 + wl
                    cw = caps[w]
                    if w < WINDOWS - BANKW:
                        bslot = w % BANKW
                        if bslot == 0:
                            pbank[0] = ps0.tile([128, 512], f32, tag="b0",
                                                name="pb0")
                            pbank[1] = ps1.tile([128, 512], f32, tag="b1",
                                                name="pb1")
                    elif w == WINDOWS - BANKW:
                        bslot = 0
                        pbank[0] = ps0.tile([128, (BANKW - 1) * KS], f32,
                                            tag="b0", name="pb0a")
                        pbank[1] = ps1.tile([128, (BANKW - 1) * KS], f32,
                                            tag="b1", name="pb1a")
                    elif w < WINDOWS - 1:
                        bslot = w % BANKW
                    else:
                        bslot = 0
                        pbank[0] = ps0.tile([128, KS], f32, tag="b0",
                                            name="pb0b")
                        pbank[1] = ps1.tile([128, KS], f32, tag="b1",
                                            name="pb1b")
                    # aggregate: psum[ci, (k,d)] += Hg_chunk.T @ rhs_chunk
                    for ch in range(cw):
                        chl = off[w] - off[w0] + ch
                        for half in range(2):
                            nc.tensor.matmul(
                                pbank[half][:, bslot * KS:(bslot + 1) * KS],
                                hg[:, chl, half * 128:(half + 1) * 128]
                                .bitcast(hg_view),
                                rhs[:, chl, :],
                                start=(ch == 0),
                                stop=(ch == cw - 1),
                            )
                    copy_lo = None
                    if w < WINDOWS - BANKW and w % BANKW == BANKW - 1:
                        copy_lo = w - BANKW + 1        # full bank
                    elif w in (WINDOWS - 2, WINDOWS - 1):
                        copy_lo = (WINDOWS - BANKW if w == WINDOWS - 2
                                   else WINDOWS - 1)   # split last bank
                    if copy_lo is not None:
                        for half in range(2):
                            dst = resstack[half][:, copy_lo * KS:
                                                 (w + 1) * KS]
                            if half == 0:
                                nc.vector.tensor_copy(dst, pbank[half][:])
                            else:
                                nc.scalar.copy(dst, pbank[half][:])
                # emit finals whose windows were all copied before this group
                while (g > WTG and fin_done < len(SEGS) - 2
                       and SEGS[fin_done][0] + SEGS[fin_done][1] <= w0):
                    emit_final(*SEGS[fin_done])
                    fin_done += 1

            nc.sync.dma_start(out_d.ap()[:, :SEGS[-2][2]],
                              out_sb[:, :SEGS[-2][2]])
            fill_pe(TAILF)
            emit_final(*SEGS[-2])
            emit_final(*SEGS[-1], add_eng="mix")
            fill_pe(0, last=True)
            nc.vector.tensor_copy(
                out_sb[:, OUT_COLS:OUT_COLS + 32].bitcast(f32),
                wps[:, :16])
            nc.sync.dma_start(out_d.ap()[:, SEGS[-2][2]:],
                              out_sb[:, SEGS[-2][2]:])

    nc.compile()
    return nc


# ------------------------------------------------------------------ host ----
def _greedy_partition(items_deg, nbins, cap):
    """Assign item ids (sorted desc by degree) to bins; each bin gets at most
    `cap` items, minimizing max degree-sum.  Returns list of lists."""
    import heapq
    bins = [[] for _ in range(nbins)]
    heap = [(0, b) for b in range(nbins)]
    heapq.heapify(heap)
    for it, dg in items_deg:
        s, b = heapq.heappop(heap)
        bins[b].append(it)
        if len(bins[b]) < cap:
            heapq.heappush(heap, (s + dg, b))
    return bins


def _pack_windows(items_deg, caps):
    """Pack (dst, deg) items into len(caps) bins of SLOTS items each with
    bin w's degree-sum <= caps[w]*128.  Returns list of lists or None."""
    nb = len(caps)
    rem_cap = [c * 128 for c in caps]
    rem_slots = [SLOTS] * nb
    bins = [[] for _ in range(nb)]
    for it, dg in items_deg:           # desc by degree
        best, best_score = -1, None
        for b in range(nb):
            if rem_slots[b] == 0 or rem_cap[b] < dg:
                continue
            score = (rem_cap[b] - dg) / rem_slots[b]
            if best_score is None or score > best_score:
                best, best_score = b, score
        if best < 0:
            return None
        bins[best].append(it)
        rem_cap[best] -= dg
        rem_slots[best] -= 1
    return bins


def _make_caps(E_core):
    """Per-window-ordinal chunk capacities.  Wide windows sit in the middle
    groups (absorbing degree-sum variance), the tail gets narrow windows so
    the last hg load covers a single window."""
    base = -(-E_core // (WINDOWS * 128))           # avg chunks per window
    outs = []
    for nwide in (1, 2, 3, 4, 6, 8, 16, 24, WINDOWS - GG):
        caps = ([base] * GG + [base + 1] * nwide
                + [base] * (WINDOWS - GG - nwide))
        outs.append(caps)
    return outs


def kernel(h, X, edge_index, batch_node, weight, bias):
    h = np.asarray(h, dtype=np.float32)
    X = np.asarray(X, dtype=np.float32)
    edge_index = np.asarray(edge_index)
    weight = np.asarray(weight, dtype=np.float32)
    bias = np.asarray(bias, dtype=np.float32)

    src = edge_index[0].astype(np.int64)
    dst = edge_index[1].astype(np.int64)
    E = src.shape[0]

    deg = np.bincount(dst, minlength=N)
    order = np.argsort(-deg, kind="stable")

    # dst -> core (8 bins of 512), then per core dst -> window
    core_bins = _greedy_partition([(int(m), int(deg[m])) for m in order],
                                  NCORES, DSTS_PER_CORE)

    # edges grouped by dst
    eorder = np.argsort(dst, kind="stable")
    starts = np.searchsorted(dst[eorder], np.arange(N))
    ends = np.searchsorted(dst[eorder], np.arange(N) + 1)

    core_windows = None
    caps = None
    for caps_try in _make_caps(E // NCORES):
        packs = []
        for c in range(NCORES):
            items = [(m, int(deg[m])) for m in
                     sorted(core_bins[c], key=lambda m: -deg[m])]
            p = _pack_windows(items, caps_try)
            if p is None:
                break
            packs.append(p)
        if len(packs) == NCORES:
            core_windows, caps = packs, caps_try
            break
    if core_windows is None:
        # fallback: uniform capacity from the worst window under plain LPT
        max_cnt = 0
        core_windows = []
        for c in range(NCORES):
            items = [(m, int(deg[m])) for m in
                     sorted(core_bins[c], key=lambda m: -deg[m])]
            wins = _greedy_partition(items, WINDOWS, SLOTS)
            core_windows.append(wins)
            for wlist in wins:
                max_cnt = max(max_cnt, int(sum(deg[m] for m in wlist)))
        caps = [max(1, -(-max_cnt // 128))] * WINDOWS

    key = (tuple(caps), SLOTS, GG, NWU, TAILF, WTG, HGDT, tuple(FILLS))
    if key not in _prog_cache:
        _prog_cache[key] = _build_program(caps)
    nc = _prog_cache[key]

    off = [0]
    for cw in caps:
        off.append(off[-1] + cw)
    nchunk = off[-1]
    # quantized h rows, [N, C]
    if HGDT == "f8":
        h8T = np.ascontiguousarray(h.T).astype(ml_dtypes.float8_e3m4) \
            .view(np.int8)
    else:
        h8T = np.ascontiguousarray(h.T).astype(np.float16).view(np.int16)
    wT = np.ascontiguousarray(
        weight.transpose(0, 2, 1).reshape(K * C, C)).astype(np.float16)
    bias2 = np.ascontiguousarray(bias.reshape(2, 128))
    iota = np.broadcast_to(np.arange(SLOTS, dtype=np.float16),
                           (128, SLOTS)).copy()

    in_maps = []
    perms = []
    for c in range(NCORES):
        hg8 = np.zeros((128, nchunk, C),
                       dtype=np.int8 if HGDT == "f8" else np.int16)
        xr = np.zeros((128, nchunk, K), dtype=np.float16)
        dl = np.zeros((128, nchunk), dtype=np.float16)
        perm = np.empty(DSTS_PER_CORE, dtype=np.int64)
        for w in range(WINDOWS):
            wl = core_windows[c][w]
            el = []
            sl = []
            for d_slot, m in enumerate(wl):
                perm[w * SLOTS + d_slot] = m
                ee = eorder[starts[m]:ends[m]]
                el.append(ee)
                sl.append(np.full(ee.shape[0], d_slot, dtype=np.float16))
            el = (np.concatenate(el) if el else
                  np.empty(0, dtype=np.int64))
            sl = (np.concatenate(sl) if sl else
                  np.empty(0, dtype=np.float16))
            so = np.argsort(src[el], kind="stable")
            el, sl = el[so], sl[so]
            L = el.shape[0]
            j = np.arange(L)
            p = j % 128
            ch = off[w] + j // 128
            xr[p, ch, :] = X[el, :]
            dl[p, ch] = sl
            hg8[p, ch, :] = h8T[src[el], :]
        pad = np.zeros((128, (nchunk * K + nchunk + SLOTS) % 2),
                       dtype=np.int16)
        epk = np.concatenate(
            [xr.reshape(128, nchunk * K).view(np.int16),
             dl.view(np.int16),
             iota.view(np.int16), pad,
             np.ascontiguousarray(bias2.T.astype(np.float32))
               .view(np.int16).reshape(128, 4)],
            axis=1)
        in_maps.append({
            "hg8": np.ascontiguousarray(hg8.reshape(128, nchunk * C)),
            "wT": wT,
            "epk": np.ascontiguousarray(epk),
        })
        perms.append(perm)

    global _last_perms
    _last_perms = perms

    try:
        res = run_bass_kernel_spmd(nc, in_maps, core_ids=list(range(NCORES)))
    except Exception:
        # transient device-state issues usually clear on retry
        time.sleep(10)
        res = run_bass_kernel_spmd(nc, in_maps, core_ids=list(range(NCORES)))

    FW = 8
    segs = [(i * FW, FW, i * 2 * FW * SLOTS)
            for i in range(WINDOWS // FW - 1)]
    segs += [(WINDOWS - FW, FW - 1, segs[-1][2] + 2 * FW * SLOTS),
             (WINDOWS - 1, 1, segs[-1][2] + 4 * FW * SLOTS - 2 * SLOTS)]
    out = np.empty((C, N), dtype=np.float32)
    for c in range(NCORES):
        raw = np.asarray(res.results[c]["out"]).astype(np.float32)
        oc = np.empty((C, DSTS_PER_CORE), dtype=np.float32)
        for w_lo, w_cnt, co in segs:
            m2 = w_cnt * SLOTS
            for oh in range(2):
                oc[oh * 128:(oh + 1) * 128,
                   w_lo * SLOTS:w_lo * SLOTS + m2] = \
                    raw[:, co + oh * m2:co + (oh + 1) * m2]
        out[:, perms[c]] = oc
    return out
